# revision 38
# baseline (speedup 1.0000x reference)
"""Trainium2 Bass kernel for a 5x5 valid convolution over 96x96 images.

Reference computes x @ W.T where W is the [8464, 9216] conv-as-matmul
matrix (10 GFLOP dense).  We compute the convolution directly on the
tensor engine as 5 PSUM-accumulated banded matmuls (row-conv over the
image-row contraction, column shifts folded into the rhs access pattern):

    out[oi, b, oj] = sum_kj  B_kj.T @ X[:, b, oj+kj]
    B_kj[i, oi]    = K[i-oi, kj]   (banded Toeplitz)

The band matrix depends only on the 25-float kernel K, so it is
expanded on the host (like the reference's conv_mat) and passed as a
per-core input.  DRAM params are declared float32r (bit-identical to
fp32) so matmul operands come straight from DMA with no DVE casts.

Schedule notes (from NTFF traces): the PE clock ramps 0.65->2.4GHz
only under sustained use, so throwaway matmuls warm it while the loads
are in flight (conv matmuls then run ~330ns instead of ~510ns).  The
band is split across both HWDGE rings ahead of the x halves; stores go
out per 2-image quarter as each PSUM half is copied, with the last
quarter on the gpsimd software DGE whose coalesced big-packet writes
relieve the four shared DRAM-write engines (the end-to-end bottleneck:
~60GB/s aggregate for the 271KB output).
"""

import sys

sys.path.insert(0, "/opt/trn_rl_repo")

import numpy as np

import bass_rust
import concourse.bass as bass
import concourse.mybir as mybir
from concourse.bass_utils import run_bass_kernel_spmd

# Problem geometry (hardcoded per the task contract).
BATCH = 64
IN = 96           # input image side
KD = 5            # conv kernel side
OD = IN - KD + 1  # output side = 92
ISIZE = IN * IN   # 9216
OSIZE = OD * OD   # 8464
NCORES = 8
BPC = BATCH // NCORES  # images per core = 8
HALF = BPC // 2        # images per PSUM accumulation group = 4
QTR = BPC // 4         # images per store quarter = 2


def _ap(view, offset, dims):
    ap = view.copy()
    ap.offset = offset
    ap.ap = bass_rust.VecI64Pair(dims)
    return ap


def _build_program():
    nc = bass.Bass()
    dt = mybir.dt.float32
    f32r = mybir.dt.float32r

    x_in = nc.declare_dram_parameter("x", [BPC, ISIZE], f32r, isOutput=False)
    b_in = nc.declare_dram_parameter("b", [IN, KD * OD], f32r, isOutput=False)
    y_out = nc.declare_dram_parameter("y", [BPC, OSIZE], dt, isOutput=True)

    from contextlib import ExitStack

    with ExitStack() as ctx:
        b_sb = ctx.enter_context(nc.sbuf_tensor("b_sb", [IN, KD, OD], f32r))
        x_sb = ctx.enter_context(nc.sbuf_tensor("x_sb", [IN, BPC, IN], f32r))
        out_sb = ctx.enter_context(nc.sbuf_tensor("out_sb", [OD, BPC, OD], dt))
        psq = [
            ctx.enter_context(nc.psum_tensor(f"ps{g}", [OD, QTR, OD], dt))
            for g in range(4)
        ]
        warm_sb = ctx.enter_context(nc.sbuf_tensor("warm_sb", [128, 512], dt))
        ps_w = ctx.enter_context(nc.psum_tensor("ps_w", [16, 512], dt))
        sem = lambda n: ctx.enter_context(nc.semaphore(n))
        sem_b = sem("sem_b")          # band matrix -> b_sb
        sem_x = [sem(f"sem_x{p}") for p in range(4)]  # image pairs
        sem_mm = sem("sem_mm")        # psum group done
        sem_cv = sem("sem_cv")        # DVE quarter copies
        sem_y = sem("sem_y")          # out_sb -> y

        sem_w = sem("sem_w")

        # ---- PE p-state warm-up: the tensor engine clock ramps from
        # 0.65GHz to 2.4GHz only after ~3us of continuous use, so run
        # throwaway matmuls on a Pool-zeroed scratch tile while the real
        # loads are still in flight.  Without this the conv matmuls all
        # execute at the cold/mid clock.
        nc.gpsimd.memset(warm_sb[:], 0).then_inc(sem_w, 1)
        nc.tensor.wait_ge(sem_w, 1)
        warm_r = warm_sb[:].bitcast(f32r)
        warm_l = warm_sb[:, 0:16].bitcast(f32r)
        for _ in range(7):
            nc.tensor.matmul(ps_w[:], warm_l, warm_r, start=True, stop=True)

        # ---- loads: band whole on sync first (small, lands early); the
        # x image-pairs alternate rings so the first matmul group's pair
        # finishes ~1us earlier than a whole 4-image half on one ring.
        # sync: b, pair0, pair2; scalar: pair1, pair3.
        def x_load(engine, p):
            engine.dma_start(
                out=x_sb[:, p * QTR : (p + 1) * QTR, :],
                in_=_ap(
                    x_in[:], p * QTR * ISIZE,
                    [[IN, IN], [ISIZE, QTR], [1, IN]],
                ),
            ).then_inc(sem_x[p], 16)

        nc.sync.dma_start(out=b_sb[:], in_=b_in[:]).then_inc(sem_b, 16)
        x_load(nc.sync, 0)
        x_load(nc.scalar, 1)
        x_load(nc.sync, 2)
        x_load(nc.scalar, 3)

        # ---- tensor: four 2-image accumulation groups (g-outer, kj
        # inner).  Warm matmuls are short (~130ns/group-kj) so finer
        # groups release the first store quarter ~1us earlier and keep
        # the DRAM-write engines continuously fed.
        nc.tensor.wait_ge(sem_b, 16)
        for g in range(4):
            nc.tensor.wait_ge(sem_x[g], 16)
            for kj in range(KD):
                mm = nc.tensor.matmul(
                    psq[g][:],
                    b_sb[:, kj, :],
                    _ap(
                        x_sb[:],
                        g * QTR * IN + kj,
                        [[BPC * IN, IN], [IN, QTR], [1, OD]],
                    ),
                    start=(kj == 0),
                    stop=(kj == KD - 1),
                )
                if kj == KD - 1:
                    mm.then_inc(sem_mm, 1)

        # ---- psum -> out_sb quarter copies (DVE) + per-quarter stores
        # spread over sync, scalar, and the gpsimd software DGE.
        def store(engine, q, s, v):
            engine.wait_ge(s, v)
            engine.dma_start(
                out=_ap(
                    y_out[:],
                    q * QTR * OSIZE,
                    [[OD, OD], [OSIZE, QTR], [1, OD]],
                ),
                in_=out_sb[:, q * QTR : (q + 1) * QTR, :],
            ).then_inc(sem_y, 16)

        for q in range(4):
            nc.vector.wait_ge(sem_mm, q + 1)
            nc.vector.tensor_copy(
                out_sb[:, q * QTR : (q + 1) * QTR, :],
                psq[q][:],
            ).then_inc(sem_cv, 1)
        store(nc.gpsimd, 0, sem_cv, 1)
        store(nc.sync, 1, sem_cv, 2)
        store(nc.scalar, 2, sem_cv, 3)
        store(nc.sync, 3, sem_cv, 4)

        # hold execution open until every store has landed
        nc.sync.wait_ge(sem_y, 64)

    return nc


def _band_matrix(k: np.ndarray) -> np.ndarray:
    """Pre-reversed banded Toeplitz: b[i, kj, oi] = K[i-oi, kj]."""
    b = np.zeros((IN, KD, OD), np.float32)
    oi = np.arange(OD)
    for t in range(KD):
        for kj in range(KD):
            b[oi + t, kj, oi] = k[t, kj]
    return b.reshape(IN, KD * OD)


_NC = None


def kernel(x: np.ndarray, kernel: np.ndarray) -> np.ndarray:
    global _NC
    if _NC is None:
        _NC = _build_program()

    x = np.ascontiguousarray(x, dtype=np.float32)
    k = np.ascontiguousarray(kernel, dtype=np.float32)
    b = _band_matrix(k)
    in_maps = [
        {"x": x[c * BPC : (c + 1) * BPC], "b": b} for c in range(NCORES)
    ]
    res = run_bass_kernel_spmd(_NC, in_maps, list(range(NCORES)))
    return np.concatenate([res.results[c]["y"] for c in range(NCORES)], axis=0)


# revision 40
# speedup vs baseline: 1.1064x; 1.1064x over previous
"""Trainium2 Bass kernel for a 5x5 valid convolution over 96x96 images.

Reference computes x @ W.T where W is the [8464, 9216] conv-as-matmul
matrix (10 GFLOP dense).  We compute the convolution directly on the
tensor engine as 5 PSUM-accumulated banded matmuls (row-conv over the
image-row contraction, column shifts folded into the rhs access pattern):

    out[oi, b, oj] = sum_kj  B_kj.T @ X[:, b, oj+kj]
    B_kj[i, oi]    = K[i-oi, kj]   (banded Toeplitz)

The band matrix depends only on the 25-float kernel K, so it is
expanded on the host (like the reference's conv_mat) and passed as a
per-core input.  DRAM params are declared float32r (bit-identical to
fp32) so matmul operands come straight from DMA with no DVE casts.

Schedule notes (from NTFF traces): the PE clock ramps 0.65->2.4GHz
only under sustained use, so throwaway matmuls warm it while the loads
are in flight (conv matmuls then run ~330ns instead of ~510ns).  The
band is split across both HWDGE rings ahead of the x halves; stores go
out per 2-image quarter as each PSUM half is copied, with the last
quarter on the gpsimd software DGE whose coalesced big-packet writes
relieve the four shared DRAM-write engines (the end-to-end bottleneck:
~60GB/s aggregate for the 271KB output).
"""

import sys

sys.path.insert(0, "/opt/trn_rl_repo")

import numpy as np

import bass_rust
import concourse.bass as bass
import concourse.mybir as mybir
from concourse.bass_utils import run_bass_kernel_spmd

# Problem geometry (hardcoded per the task contract).
BATCH = 64
IN = 96           # input image side
KD = 5            # conv kernel side
OD = IN - KD + 1  # output side = 92
ISIZE = IN * IN   # 9216
OSIZE = OD * OD   # 8464
NCORES = 8
BPC = BATCH // NCORES  # images per core = 8
HALF = BPC // 2        # images per PSUM accumulation group = 4
QTR = BPC // 4         # images per store quarter = 2


def _ap(view, offset, dims):
    ap = view.copy()
    ap.offset = offset
    ap.ap = bass_rust.VecI64Pair(dims)
    return ap


def _build_program():
    nc = bass.Bass()
    dt = mybir.dt.float32
    f32r = mybir.dt.float32r

    x_in = nc.declare_dram_parameter("x", [BPC, ISIZE], f32r, isOutput=False)
    b_in = nc.declare_dram_parameter("b", [IN, KD * OD], f32r, isOutput=False)
    y_out = nc.declare_dram_parameter("y", [BPC, OSIZE], dt, isOutput=True)

    from contextlib import ExitStack

    with ExitStack() as ctx:
        b_sb = ctx.enter_context(nc.sbuf_tensor("b_sb", [IN, KD, OD], f32r))
        x_sb = ctx.enter_context(nc.sbuf_tensor("x_sb", [IN, BPC, IN], f32r))
        out_sb = ctx.enter_context(nc.sbuf_tensor("out_sb", [OD, BPC, OD], dt))
        psq = [
            ctx.enter_context(nc.psum_tensor(f"ps{g}", [OD, QTR, OD], dt))
            for g in range(4)
        ]
        warm_sb = ctx.enter_context(nc.sbuf_tensor("warm_sb", [128, 512], dt))
        ps_w = ctx.enter_context(nc.psum_tensor("ps_w", [16, 512], dt))
        sem = lambda n: ctx.enter_context(nc.semaphore(n))
        sem_b = sem("sem_b")          # band matrix -> b_sb
        sem_x = [sem(f"sem_x{p}") for p in range(4)]  # image pairs
        sem_mm = sem("sem_mm")        # psum group done
        sem_cv = sem("sem_cv")        # DVE quarter copies
        sem_y = sem("sem_y")          # out_sb -> y

        sem_w = sem("sem_w")

        # ---- PE p-state warm-up: the tensor engine clock ramps from
        # 0.65GHz to 2.4GHz only after ~3us of continuous use, so run
        # throwaway matmuls on a Pool-zeroed scratch tile while the real
        # loads are still in flight.  Without this the conv matmuls all
        # execute at the cold/mid clock.
        nc.gpsimd.memset(warm_sb[:], 0).then_inc(sem_w, 1)
        nc.tensor.wait_ge(sem_w, 1)
        warm_r = warm_sb[:].bitcast(f32r)
        warm_l = warm_sb[:, 0:16].bitcast(f32r)
        for _ in range(7):
            nc.tensor.matmul(ps_w[:], warm_l, warm_r, start=True, stop=True)

        # ---- loads: band whole on sync first (small, lands early); the
        # x image-pairs alternate rings so the first matmul group's pair
        # finishes ~1us earlier than a whole 4-image half on one ring.
        # sync: b, pair0, pair2; scalar: pair1, pair3.
        def x_load(engine, p):
            engine.dma_start(
                out=x_sb[:, p * QTR : (p + 1) * QTR, :],
                in_=_ap(
                    x_in[:], p * QTR * ISIZE,
                    [[IN, IN], [ISIZE, QTR], [1, IN]],
                ),
            ).then_inc(sem_x[p], 16)

        nc.sync.dma_start(out=b_sb[:], in_=b_in[:]).then_inc(sem_b, 16)
        x_load(nc.sync, 0)
        x_load(nc.scalar, 1)
        x_load(nc.sync, 2)
        x_load(nc.scalar, 3)

        # ---- tensor: four 2-image accumulation groups (g-outer, kj
        # inner).  Warm matmuls are short (~130ns/group-kj) so finer
        # groups release the first store quarter ~1us earlier and keep
        # the DRAM-write engines continuously fed.
        nc.tensor.wait_ge(sem_b, 16)
        # pair1 (scalar ring's first DMA) completes before pair0 (queued
        # behind the band on sync), so run its group first
        for g in (1, 0, 2, 3):
            nc.tensor.wait_ge(sem_x[g], 16)
            for kj in range(KD):
                mm = nc.tensor.matmul(
                    psq[g][:],
                    b_sb[:, kj, :],
                    _ap(
                        x_sb[:],
                        g * QTR * IN + kj,
                        [[BPC * IN, IN], [IN, QTR], [1, OD]],
                    ),
                    start=(kj == 0),
                    stop=(kj == KD - 1),
                )
                if kj == KD - 1:
                    mm.then_inc(sem_mm, 1)

        # ---- psum -> out_sb quarter copies (DVE) + per-quarter stores
        # spread over sync, scalar, and the gpsimd software DGE.
        def store(engine, q, s, v):
            engine.wait_ge(s, v)
            engine.dma_start(
                out=_ap(
                    y_out[:],
                    q * QTR * OSIZE,
                    [[OD, OD], [OSIZE, QTR], [1, OD]],
                ),
                in_=out_sb[:, q * QTR : (q + 1) * QTR, :],
            ).then_inc(sem_y, 16)

        for i, p in enumerate((1, 0, 2, 3)):
            nc.vector.wait_ge(sem_mm, i + 1)
            nc.vector.tensor_copy(
                out_sb[:, p * QTR : (p + 1) * QTR, :],
                psq[p][:],
            ).then_inc(sem_cv, 1)
        store(nc.sync, 1, sem_cv, 1)
        store(nc.gpsimd, 0, sem_cv, 2)
        store(nc.scalar, 2, sem_cv, 3)
        store(nc.sync, 3, sem_cv, 4)

        # hold execution open until every store has landed
        nc.sync.wait_ge(sem_y, 64)

    return nc


def _band_matrix(k: np.ndarray) -> np.ndarray:
    """Pre-reversed banded Toeplitz: b[i, kj, oi] = K[i-oi, kj]."""
    b = np.zeros((IN, KD, OD), np.float32)
    oi = np.arange(OD)
    for t in range(KD):
        for kj in range(KD):
            b[oi + t, kj, oi] = k[t, kj]
    return b.reshape(IN, KD * OD)


_NC = None


def kernel(x: np.ndarray, kernel: np.ndarray) -> np.ndarray:
    global _NC
    if _NC is None:
        _NC = _build_program()

    x = np.ascontiguousarray(x, dtype=np.float32)
    k = np.ascontiguousarray(kernel, dtype=np.float32)
    b = _band_matrix(k)
    in_maps = [
        {"x": x[c * BPC : (c + 1) * BPC], "b": b} for c in range(NCORES)
    ]
    res = run_bass_kernel_spmd(_NC, in_maps, list(range(NCORES)))
    return np.concatenate([res.results[c]["y"] for c in range(NCORES)], axis=0)
